# revision 13
# baseline (speedup 1.0000x reference)
"""Single-head causal attention (B=4, T=4096, C=1024, H=128) on 8 NeuronCores.

v5 "kv-interleaved flash split, sub-block wavefront": 2 cores per batch.
Within a batch, core role r owns the kv tiles with index ≡ r (mod 2)
(16 of 32 tiles of 128 keys). Each core projects K/V only for its own
tiles and Q for the full batch, then computes flash-style partial
attention of ALL 4096 queries against its kv half. The host merges all
partials exactly:  out = sum(N_partials) / sum(d_partials).

SPMD trick: the host permutes x columns per core with a pair-swap
(tile index XOR role), so each core's own kv tiles sit at even 128-col
slots. q-block p (= slots 2p, 2p+1) then needs exactly p+1 own kv
tiles on either role — the device program is identical across cores,
with zero trip padding. The causal boundary reduces to one data-driven
[tri | ones-or-zeros] mask applied to the last trip of each block.

Blocks 8..15 are split into two independent sub-blocks (trips 0..7 and
8..p) with separate PSUM accumulators and separate DMA'd partials, so
their exp/U work overlaps the projection phase instead of piling into
an ACT-bound tail (each engine's work is roughly level across the
kernel). Device emission follows the x-DMA wavefront.

Device program (per core, all matmuls bf16 with f32 PSUM):
  K^T tiles (pos j) = Wk^T.T @ x_own ; V tiles directly via
    lhsT=x_own-chunk (out [kv,h]; no PE transposes anywhere)
  Q^T = (Wq/sqrt(H))^T.T @ x   (full batch)
  per sub-block (256 q, trips j0..j1, groups of <=4 kv tiles):
    S^T[kv,q] = K_j^T.T @ Q^T ; P = exp(S^T) (one ACT instr per group);
    mask last causal trip ; U (+)= P pairs (bf16, DVE) ;
    N^T[h,q] += V_j.T @ P
  DMA out N^T (f32) and U (bf16) raw; host does colsum/divide/merge/
  transpose (flash, no running max: logits bounded for this input
  distribution; bf16 U is exact enough: ~0.1% relative on d).
"""
import os
import sys

import numpy as np

try:
    import ml_dtypes
except ImportError:  # pragma: no cover
    sys.path.insert(0, "/opt/trn_rl_repo")
    import ml_dtypes

for _p in ("/opt/trn_rl_repo",):
    if os.path.isdir(_p) and _p not in sys.path:
        sys.path.insert(0, _p)

try:
    import jax as _jax
    _jax.config.update("jax_compilation_cache_dir", "/tmp/jax_neff_cache")
    _jax.config.update("jax_persistent_cache_min_entry_size_bytes", -1)
    _jax.config.update("jax_persistent_cache_min_compile_time_secs", 0.0)
except Exception:
    pass

import concourse.bass as bass
import concourse.mybir as mybir
import concourse.tile as tile
from concourse import bacc
from concourse.bass_utils import run_bass_kernel_spmd

B, T, C, H = 4, 4096, 1024, 128
P = 128            # partitions / tile edge
CK = C // P        # 8 contraction chunks
QW = 256           # q-block width (2 subtiles)
NBLK = T // QW     # 16 q-blocks per core (full batch)
NPOS = 16          # own kv tiles per core
BF16 = ml_dtypes.bfloat16
SCALE = float(np.sqrt(H))

# attention sub-blocks: (block p, first trip j0, end trip j1, output slot)
SUBBLOCKS = []
for _p_ in range(8):
    SUBBLOCKS.append((_p_, 0, _p_ + 1))
for _p_ in range(8, NBLK):
    SUBBLOCKS.append((_p_, 0, 8))
for _p_ in range(8, NBLK):
    SUBBLOCKS.append((_p_, 8, _p_ + 1))
NSUB = len(SUBBLOCKS)          # 24
_SLOT = {(p, j0): i for i, (p, j0, _) in enumerate(SUBBLOCKS)}

_prog_cache = {}


def _build_program(loop_n=None, loads_in_loop=True) -> bass.Bass:
    nc = bacc.Bacc("TRN2")
    dt = mybir.dt
    f32, bf16 = dt.float32, dt.bfloat16

    xT_d = nc.declare_dram_parameter("xT", [C, T], dt.bfloat16, isOutput=False)
    w_d = nc.declare_dram_parameter("w_all", [C, 3 * H], dt.bfloat16, isOutput=False)
    mask_d = nc.declare_dram_parameter("masks", [P, QW], dt.bfloat16, isOutput=False)
    outT_d = nc.declare_dram_parameter("outT", [P, NSUB * QW], dt.float32, isOutput=True)
    u_d = nc.declare_dram_parameter("u", [P, NSUB * 2 * QW], dt.bfloat16, isOutput=True)

    with tile.TileContext(nc) as tc:
        with (
            tc.tile_pool(name="consts", bufs=1) as consts,
            tc.tile_pool(name="bigx", bufs=1) as bigx,
            tc.tile_pool(name="persist", bufs=1) as persist,
            tc.tile_pool(name="psum_proj", bufs=2, space="PSUM") as psum_proj,
            tc.tile_pool(name="psum_s", bufs=2, space="PSUM") as psum_s,
            tc.tile_pool(name="psum_o", bufs=2, space="PSUM") as psum_o,
            tc.tile_pool(name="sb_p", bufs=4) as sb_p,
            tc.tile_pool(name="sb_u", bufs=4) as sb_u,
            tc.tile_pool(name="sb_o", bufs=4) as sb_o,
        ):
            import contextlib

            def loop_or_null(active):
                return tc.For_i(0, loop_n, 1) if (loop_n and active) else contextlib.nullcontext()

            with loop_or_null(loads_in_loop):
                # ---- constants ----
                w_sb = consts.tile([P, CK * 3 * H], bf16, tag="w")
                masks_sb = consts.tile([P, QW], bf16, tag="masks")

                def wq_s(ck):
                    return w_sb[:, ck * 3 * H: ck * 3 * H + H]

                def wk_s(ck):
                    return w_sb[:, ck * 3 * H + H: ck * 3 * H + 2 * H]

                def wv_s(ck):
                    return w_sb[:, ck * 3 * H + 2 * H: ck * 3 * H + 3 * H]

                # ---- stream inputs (issue order = consumption order) ----
                x_sb = bigx.tile([P, CK * T], bf16, tag="x")
                x3 = x_sb[:].rearrange("p (ck t) -> p ck t", t=T)
                xd3 = xT_d.ap().rearrange("(ck p) t -> p ck t", p=P)

                w3 = w_sb[:].rearrange("p (ck h) -> p ck h", h=3 * H)
                wd3 = w_d.ap().rearrange("(ck p) h -> p ck h", p=P)
                nc.sync.dma_start(w3[:, 0:1, :], wd3[:, 0:1, :])
                nc.sync.dma_start(w3[:, 1:, :], wd3[:, 1:, :])
                nc.scalar.dma_start(x3[:, :, 0:512], xd3[:, :, 0:512])
                nc.scalar.dma_start(x3[:, :, 512:1024], xd3[:, :, 512:1024])
                nc.sync.dma_start(x3[:, :, 1024:2048], xd3[:, :, 1024:2048])
                nc.scalar.dma_start(masks_sb[:], mask_d.ap()[:])
                nc.scalar.dma_start(x3[:, :, 2048:3072], xd3[:, :, 2048:3072])
                nc.sync.dma_start(x3[:, :, 3072:4096], xd3[:, :, 3072:4096])

                kT_sb = persist.tile([P, NPOS * P], bf16, tag="kT")
                v_sb = persist.tile([P, NPOS * H], bf16, tag="v")
                qT_sb = persist.tile([P, T], bf16, tag="qT")

                with loop_or_null(not loads_in_loop):

                    def emit_kv_group(g):
                        """K^T and V for own positions 2g, 2g+1 (x cols 512g..+512)."""
                        ps = psum_proj.tile([P, 4 * P], f32, tag="proj")
                        for ck in range(CK):
                            base = ck * T + 512 * g
                            rhs = (
                                x_sb[:, base: base + 512]
                                .rearrange("p (f s t) -> p f s t", f=2, s=2)
                                [:, :, 0:1, :]
                            )
                            nc.tensor.matmul(
                                ps[:, 0: 2 * P], lhsT=wk_s(ck), rhs=rhs,
                                start=(ck == 0), stop=(ck == CK - 1),
                            )
                        nc.vector.tensor_scalar_mul(
                            kT_sb[:, 2 * P * g: 2 * P * (g + 1)], ps[:, 0: 2 * P], 1.0)

                        pv = psum_proj.tile([P, 4 * P], f32, tag="proj")
                        for u in range(2):
                            pos = 2 * g + u
                            for ck in range(CK):
                                nc.tensor.matmul(
                                    pv[:, u * H:(u + 1) * H],
                                    lhsT=x_sb[:, ck * T + 2 * P * pos: ck * T + 2 * P * pos + P],
                                    rhs=wv_s(ck),
                                    start=(ck == 0), stop=(ck == CK - 1),
                                )
                        nc.vector.tensor_scalar_mul(
                            v_sb[:, 2 * g * H: 2 * (g + 1) * H], pv[:, 0: 2 * H], 1.0)

                    def emit_q(t):
                        """Q^T for blocks 2t, 2t+1 (x cols 512t..+512)."""
                        ps = psum_proj.tile([P, 4 * P], f32, tag="proj")
                        for ck in range(CK):
                            nc.tensor.matmul(
                                ps[:], lhsT=wq_s(ck),
                                rhs=x_sb[:, ck * T + 512 * t: ck * T + 512 * (t + 1)],
                                start=(ck == 0), stop=(ck == CK - 1),
                            )
                        nc.vector.tensor_scalar_mul(
                            qT_sb[:, 512 * t: 512 * (t + 1)], ps[:], 1.0)

                    def emit_attn(p, j0, j1):
                        slot = _SLOT[(p, j0)]
                        trips = j1 - j0
                        qs = qT_sb[:, QW * p: QW * (p + 1)]
                        po = psum_o.tile([P, QW], f32, tag="po")
                        U = sb_u.tile([P, 2 * QW], bf16, tag="U")
                        first_u = True
                        ngroups = (trips + 3) // 4
                        for g in range(ngroups):
                            gt = min(4, trips - 4 * g)
                            s_ps = psum_s.tile([P, 4 * QW], f32, tag="s")
                            for u in range(gt):
                                j = j0 + 4 * g + u
                                nc.tensor.matmul(
                                    s_ps[:, QW * u: QW * (u + 1)],
                                    lhsT=kT_sb[:, P * j: P * (j + 1)],
                                    rhs=qs, start=True, stop=True,
                                )
                            pb = sb_p.tile([P, 4 * QW], bf16, tag="p")
                            nc.scalar.activation(
                                pb[:, : QW * gt], s_ps[:, : QW * gt],
                                mybir.ActivationFunctionType.Exp,
                            )
                            if 4 * g <= p - j0 < 4 * g + gt:
                                u = p - j0 - 4 * g
                                nc.vector.tensor_mul(
                                    pb[:, QW * u: QW * (u + 1)],
                                    pb[:, QW * u: QW * (u + 1)],
                                    masks_sb[:],
                                )
                            for pair0 in range(0, gt, 2):
                                width = QW * min(2, gt - pair0)
                                src = pb[:, QW * pair0: QW * pair0 + width]
                                dst = U[:, 0:width]
                                if first_u:
                                    nc.vector.tensor_copy(dst, src)
                                    first_u = False
                                else:
                                    nc.vector.tensor_add(dst, dst, src)
                            for u in range(gt):
                                j = j0 + 4 * g + u
                                nc.tensor.matmul(
                                    po[:],
                                    lhsT=v_sb[:, H * j: H * (j + 1)],
                                    rhs=pb[:, QW * u: QW * (u + 1)],
                                    start=(j == j0), stop=(j == j1 - 1),
                                )
                        oT = sb_o.tile([P, QW], f32, tag="oT")
                        nc.vector.tensor_scalar_mul(oT[:], po[:], 1.0)
                        nc.sync.dma_start(outT_d.ap()[:, QW * slot: QW * (slot + 1)], oT[:])
                        nc.scalar.dma_start(u_d.ap()[:, 2 * QW * slot: 2 * QW * (slot + 1)], U[:])

                    # ---- wavefront emission ----
                    # kv groups / q-projections follow the x sub-chunk stream;
                    # attention sub-blocks are emitted as soon as their kv
                    # prefix and qT exist, biggest-feasible-first so exp/U
                    # work stays level across the kernel.
                    emit_kv_group(0)
                    emit_q(0)
                    emit_attn(0, 0, 1)
                    emit_kv_group(1)
                    emit_q(1)
                    emit_attn(1, 0, 2)
                    emit_attn(2, 0, 3)
                    emit_kv_group(2)
                    emit_q(2)
                    emit_attn(3, 0, 4)
                    emit_attn(4, 0, 5)
                    emit_kv_group(3)
                    emit_q(3)
                    emit_attn(5, 0, 6)
                    emit_attn(6, 0, 7)
                    emit_attn(7, 0, 8)
                    for g in range(4, 8):
                        emit_kv_group(g)
                        emit_q(g)
                        emit_attn(2 * g, 0, 8)          # phi0 of blocks 8..15
                        emit_attn(2 * g + 1, 0, 8)
                        emit_attn(2 * g, 8, 2 * g + 1)  # phi1, interleaved
                        emit_attn(2 * g + 1, 8, 2 * g + 2)
    nc.compile()
    return nc


def _perm(role):
    """Per-core tile permutation: slot i holds x tile perm[i] (involution)."""
    idx = np.arange(T // P)
    return idx ^ role


def _make_core_inputs(x, Wq, Wk, Wv):
    w_all = np.concatenate([Wq.T / SCALE, Wk.T, Wv.T], axis=1)  # [C, 3H]
    w_all = np.ascontiguousarray(w_all).astype(BF16)
    tri = np.triu(np.ones((P, P), np.float32))
    in_maps = []
    for c in range(8):
        b, r = c // 2, c % 2
        rows = (np.arange(T).reshape(T // P, P)[_perm(r)]).ravel()
        xT = np.ascontiguousarray(x[b][rows].T).astype(BF16)
        masks = np.empty((P, QW), np.float32)
        masks[:, :P] = tri
        masks[:, P:] = 1.0 if r == 0 else 0.0
        in_maps.append(dict(xT=xT, w_all=w_all, masks=masks.astype(BF16)))
    return in_maps


def _merge_outputs(res):
    """Host epilogue: sum sub-block partials, unswap q order, divide."""
    full = np.empty((B, T, H), np.float32)
    for b in range(B):
        num = np.zeros((T, H), np.float32)
        den = np.zeros((T,), np.float32)
        for r in range(2):
            out = res[2 * b + r]
            nT = np.asarray(out["outT"], np.float32)   # [H, NSUB*QW]
            u = np.asarray(out["u"], np.float32)       # [P, NSUB*2*QW]
            cn = np.zeros((T, H), np.float32)
            cd = np.zeros((T,), np.float32)
            for slot, (p, j0, j1) in enumerate(SUBBLOCKS):
                sl = np.s_[QW * p: QW * (p + 1)]
                cn[sl] += nT[:, QW * slot: QW * (slot + 1)].T
                ub = u[:, 2 * QW * slot: 2 * QW * (slot + 1)]
                dp = ub[:, :QW].sum(0)
                if j1 - j0 >= 2:
                    dp = dp + ub[:, QW:].sum(0)
                cd[sl] += dp
            pm = _perm(r)
            num += cn.reshape(T // P, P, H)[pm].reshape(T, H)
            den += cd.reshape(T // P, P)[pm].reshape(T)
        full[b] = num / den[:, None]
    return full


def kernel(x, Wq, Wk, Wv):
    x = np.asarray(x, dtype=np.float32)
    if "nc" not in _prog_cache:
        _prog_cache["nc"] = _build_program()
    nc = _prog_cache["nc"]
    in_maps = _make_core_inputs(
        x, np.asarray(Wq, np.float32), np.asarray(Wk, np.float32),
        np.asarray(Wv, np.float32)
    )
    res = run_bass_kernel_spmd(nc, in_maps, list(range(8))).results
    return _merge_outputs(res)


def _mock_device(in_map):
    """Numpy emulation of the device program (fp32; validates indexing)."""
    xT = np.asarray(in_map["xT"], np.float32)       # [C, T] permuted
    w = np.asarray(in_map["w_all"], np.float32)     # [C, 3H]
    masks = np.asarray(in_map["masks"], np.float32)  # [P, QW]
    wq, wk, wv = w[:, :H], w[:, H:2 * H], w[:, 2 * H:]
    qT = wq.T @ xT                                   # [H, T]
    kT = np.concatenate(
        [wk.T @ xT[:, 2 * P * pos: 2 * P * pos + P] for pos in range(NPOS)], axis=1)
    v = np.concatenate(
        [xT[:, 2 * P * pos: 2 * P * pos + P].T @ wv for pos in range(NPOS)], axis=0
    ).reshape(NPOS, P, H)
    outT = np.zeros((H, NSUB * QW), np.float32)
    u_out = np.zeros((P, NSUB * 2 * QW), np.float32)
    for slot, (p, j0, j1) in enumerate(SUBBLOCKS):
        qs = qT[:, QW * p: QW * (p + 1)]
        po = np.zeros((H, QW), np.float32)
        U = np.zeros((P, 2 * QW), np.float32)
        for j in range(j0, j1):
            sT = kT[:, P * j: P * (j + 1)].T @ qs    # [kv, q]
            pj = np.exp(sT)
            if j == p:
                pj = pj * masks
            U[:, QW * ((j - j0) % 2): QW * ((j - j0) % 2) + QW] += pj
            po += v[j].T @ pj
        outT[:, QW * slot: QW * (slot + 1)] = po
        u_out[:, 2 * QW * slot: 2 * QW * (slot + 1)] = U
    return dict(outT=outT, u=u_out)


def _mock_check():
    rng = np.random.default_rng(0)
    x = rng.standard_normal((B, T, C)).astype(np.float32)
    s = 1.0 / np.sqrt(C)
    Wq = rng.uniform(-s, s, (H, C)).astype(np.float32)
    Wk = rng.uniform(-s, s, (H, C)).astype(np.float32)
    Wv = rng.uniform(-s, s, (H, C)).astype(np.float32)
    exp = np.empty((B, T, H), np.float32)
    causal = np.tril(np.ones((T, T), bool))
    for b in range(B):
        q = x[b] @ Wq.T
        k = x[b] @ Wk.T
        vv = x[b] @ Wv.T
        sc = (q @ k.T) / SCALE
        sc = np.where(causal, sc, -np.inf)
        sc = sc - sc.max(1, keepdims=True)
        a = np.exp(sc)
        a /= a.sum(1, keepdims=True)
        exp[b] = a @ vv
    in_maps = _make_core_inputs(x, Wq, Wk, Wv)
    res = [_mock_device(m) for m in in_maps]
    act = _merge_outputs(res)
    rel = np.linalg.norm(act - exp) / np.linalg.norm(exp)
    print(f"mock rel err: {rel:.4e}  max abs: {np.abs(act - exp).max():.3e}")
    assert rel < 2e-2, "mock check failed"


if __name__ == "__main__":
    if "--mock" in sys.argv:
        _mock_check()
    else:
        nc = _build_program()
        print("program built ok")


# revision 14
# speedup vs baseline: 1.2982x; 1.2982x over previous
"""Single-head causal attention (B=4, T=4096, C=1024, H=128) on 8 NeuronCores.

v4 "kv-interleaved flash split": 2 cores per batch. Within a batch, core
role r owns the kv tiles with index ≡ r (mod 2) (16 of 32 tiles of 128
keys). Each core projects K/V only for its own tiles (halving the K/V
projection duplication of a q-split scheme) and Q for the full batch,
then computes flash-style partial attention of ALL 4096 queries against
its kv half. The host merges the two partials exactly:
  out = (N_r0 + N_r1) / (d_r0 + d_r1).

SPMD trick: the host permutes x columns per core with a pair-swap
(tile index XOR role), so each core's own kv tiles sit at even 128-col
slots. q-block p (= slots 2p, 2p+1) then needs exactly p+1 own kv
tiles on either role — the device program is identical across cores,
with zero trip padding. The causal boundary reduces to one data-driven
[tri | ones-or-zeros] mask applied to the last trip of each block.

Device program (per core, all matmuls bf16 with f32 PSUM):
  K^T tiles (pos j) = Wk^T.T @ x_own ; V tiles directly via
    lhsT=x_own-chunk (out [kv,h]; no PE transposes anywhere)
  Q^T = (Wq/sqrt(H))^T.T @ x   (full batch)
  per q-block p (256 q, trips = p+1, groups of <=4 kv tiles):
    S^T[kv,q] = K_j^T.T @ Q^T ; P = exp(S^T) (one ACT instr per group);
    mask last trip ; U (+)= P pairs (bf16, DVE) ; N^T[h,q] += V_j.T @ P
  DMA out N^T (f32) and U (bf16) raw; host does colsum/divide/transpose
  (flash, no running max: logits bounded for this input distribution;
  bf16 U costs ~0.1% relative on d).

A finer-grained variant (sub-block split of blocks 8..15 + wavefront
emission) simulated faster on TimelineSim (90us vs 101us) but measured
134us vs 80us on HW — the extra instructions/sync saturate the NX
sequencers, which the cost model under-weights. Keeping the coarser
schedule.
"""
import os
import sys

import numpy as np

try:
    import ml_dtypes
except ImportError:  # pragma: no cover
    sys.path.insert(0, "/opt/trn_rl_repo")
    import ml_dtypes

for _p in ("/opt/trn_rl_repo",):
    if os.path.isdir(_p) and _p not in sys.path:
        sys.path.insert(0, _p)

try:
    import jax as _jax
    _jax.config.update("jax_compilation_cache_dir", "/tmp/jax_neff_cache")
    _jax.config.update("jax_persistent_cache_min_entry_size_bytes", -1)
    _jax.config.update("jax_persistent_cache_min_compile_time_secs", 0.0)
except Exception:
    pass

import concourse.bass as bass
import concourse.mybir as mybir
import concourse.tile as tile
from concourse import bacc
from concourse.bass_utils import run_bass_kernel_spmd

B, T, C, H = 4, 4096, 1024, 128
P = 128            # partitions / tile edge
CK = C // P        # 8 contraction chunks
QW = 256           # q-block width (2 subtiles)
NBLK = T // QW     # 16 q-blocks per core (full batch)
NPOS = 16          # own kv tiles per core
BF16 = ml_dtypes.bfloat16
SCALE = float(np.sqrt(H))

_prog_cache = {}


def _build_program(loop_n=None, loads_in_loop=True) -> bass.Bass:
    nc = bacc.Bacc("TRN2")
    dt = mybir.dt
    f32, bf16 = dt.float32, dt.bfloat16

    xT_d = nc.declare_dram_parameter("xT", [C, T], dt.bfloat16, isOutput=False)
    w_d = nc.declare_dram_parameter("w_all", [C, 3 * H], dt.bfloat16, isOutput=False)
    mask_d = nc.declare_dram_parameter("masks", [P, QW], dt.bfloat16, isOutput=False)
    outT_d = nc.declare_dram_parameter("outT", [P, T], dt.float32, isOutput=True)
    u_d = nc.declare_dram_parameter("u", [P, NBLK * 2 * QW], dt.bfloat16, isOutput=True)

    with tile.TileContext(nc) as tc:
        with (
            tc.tile_pool(name="consts", bufs=1) as consts,
            tc.tile_pool(name="bigx", bufs=1) as bigx,
            tc.tile_pool(name="persist", bufs=1) as persist,
            tc.tile_pool(name="psum_proj", bufs=2, space="PSUM") as psum_proj,
            tc.tile_pool(name="psum_s", bufs=2, space="PSUM") as psum_s,
            tc.tile_pool(name="psum_o", bufs=2, space="PSUM") as psum_o,
            tc.tile_pool(name="sb_p", bufs=3) as sb_p,
            tc.tile_pool(name="sb_u", bufs=3) as sb_u,
            tc.tile_pool(name="sb_o", bufs=3) as sb_o,
        ):
            import contextlib

            def loop_or_null(active):
                return tc.For_i(0, loop_n, 1) if (loop_n and active) else contextlib.nullcontext()

            with loop_or_null(loads_in_loop):
                # ---- constants ----
                w_sb = consts.tile([P, CK * 3 * H], bf16, tag="w")
                masks_sb = consts.tile([P, QW], bf16, tag="masks")

                def wq_s(ck):
                    return w_sb[:, ck * 3 * H: ck * 3 * H + H]

                def wk_s(ck):
                    return w_sb[:, ck * 3 * H + H: ck * 3 * H + 2 * H]

                def wv_s(ck):
                    return w_sb[:, ck * 3 * H + 2 * H: ck * 3 * H + 3 * H]

                # ---- stream inputs (issue order = consumption order) ----
                x_sb = bigx.tile([P, CK * T], bf16, tag="x")
                x3 = x_sb[:].rearrange("p (ck t) -> p ck t", t=T)
                xd3 = xT_d.ap().rearrange("(ck p) t -> p ck t", p=P)

                nc.sync.dma_start(
                    w_sb[:].rearrange("p (ck h) -> p ck h", h=3 * H),
                    w_d.ap().rearrange("(ck p) h -> p ck h", p=P),
                )
                nc.scalar.dma_start(masks_sb[:], mask_d.ap()[:])
                TQ = T // 4
                for j4 in range(4):
                    eng = nc.sync if j4 % 2 == 0 else nc.scalar
                    eng.dma_start(
                        x3[:, :, j4 * TQ:(j4 + 1) * TQ],
                        xd3[:, :, j4 * TQ:(j4 + 1) * TQ],
                    )

                kT_sb = persist.tile([P, NPOS * P], bf16, tag="kT")
                v_sb = persist.tile([P, NPOS * H], bf16, tag="v")
                qT_sb = persist.tile([P, T], bf16, tag="qT")

                with loop_or_null(not loads_in_loop):

                    def emit_kv_chunk(j):
                        """K^T and V for own-kv positions 4j..4j+3 (x cols 1024j..+1024)."""
                        ps = psum_proj.tile([P, 4 * P], f32, tag="proj")
                        for ck in range(CK):
                            base = ck * T + 1024 * j
                            rhs = (
                                x_sb[:, base: base + 1024]
                                .rearrange("p (f s t) -> p f s t", f=4, s=2)
                                [:, :, 0:1, :]
                            )
                            nc.tensor.matmul(
                                ps[:], lhsT=wk_s(ck), rhs=rhs,
                                start=(ck == 0), stop=(ck == CK - 1),
                            )
                        nc.vector.tensor_scalar_mul(
                            kT_sb[:, 4 * P * j: 4 * P * (j + 1)], ps[:], 1.0)

                        for half in range(2):
                            pv = psum_proj.tile([P, 2 * H], f32, tag="proj")
                            for u in range(2):
                                pos = 4 * j + 2 * half + u
                                for ck in range(CK):
                                    nc.tensor.matmul(
                                        pv[:, u * H:(u + 1) * H],
                                        lhsT=x_sb[:, ck * T + 2 * P * pos: ck * T + 2 * P * pos + P],
                                        rhs=wv_s(ck),
                                        start=(ck == 0), stop=(ck == CK - 1),
                                    )
                            nc.vector.tensor_scalar_mul(
                                v_sb[:, (4 * j + 2 * half) * H: (4 * j + 2 * half + 2) * H],
                                pv[:], 1.0)

                    def emit_q(t):
                        """Q^T for blocks 2t, 2t+1 (x cols 512t..+512)."""
                        ps = psum_proj.tile([P, 4 * P], f32, tag="proj")
                        for ck in range(CK):
                            nc.tensor.matmul(
                                ps[:], lhsT=wq_s(ck),
                                rhs=x_sb[:, ck * T + 512 * t: ck * T + 512 * (t + 1)],
                                start=(ck == 0), stop=(ck == CK - 1),
                            )
                        nc.vector.tensor_scalar_mul(
                            qT_sb[:, 512 * t: 512 * (t + 1)], ps[:], 1.0)

                    def emit_attn(p):
                        trips = p + 1
                        qs = qT_sb[:, QW * p: QW * (p + 1)]
                        po = psum_o.tile([P, QW], f32, tag="po")
                        U = sb_u.tile([P, 2 * QW], bf16, tag="U")
                        first_u = True
                        ngroups = (trips + 3) // 4
                        for g in range(ngroups):
                            gt = min(4, trips - 4 * g)
                            s_ps = psum_s.tile([P, 4 * QW], f32, tag="s")
                            for u in range(gt):
                                j = 4 * g + u
                                nc.tensor.matmul(
                                    s_ps[:, QW * u: QW * (u + 1)],
                                    lhsT=kT_sb[:, P * j: P * (j + 1)],
                                    rhs=qs, start=True, stop=True,
                                )
                            pb = sb_p.tile([P, 4 * QW], bf16, tag="p")
                            nc.scalar.activation(
                                pb[:, : QW * gt], s_ps[:, : QW * gt],
                                mybir.ActivationFunctionType.Exp,
                            )
                            if 4 * g <= p < 4 * g + gt:
                                u = p - 4 * g
                                nc.gpsimd.tensor_mul(
                                    pb[:, QW * u: QW * (u + 1)],
                                    pb[:, QW * u: QW * (u + 1)],
                                    masks_sb[:],
                                )
                            for pair0 in range(0, gt, 2):
                                width = QW * min(2, gt - pair0)
                                src = pb[:, QW * pair0: QW * pair0 + width]
                                dst = U[:, 0:width]
                                if first_u:
                                    nc.vector.tensor_copy(dst, src)
                                    first_u = False
                                else:
                                    nc.vector.tensor_add(dst, dst, src)
                            for u in range(gt):
                                j = 4 * g + u
                                nc.tensor.matmul(
                                    po[:],
                                    lhsT=v_sb[:, H * j: H * (j + 1)],
                                    rhs=pb[:, QW * u: QW * (u + 1)],
                                    start=(j == 0), stop=(j == trips - 1),
                                )
                        oT = sb_o.tile([P, QW], f32, tag="oT")
                        nc.vector.tensor_scalar_mul(oT[:], po[:], 1.0)
                        nc.sync.dma_start(outT_d.ap()[:, QW * p: QW * (p + 1)], oT[:])
                        nc.scalar.dma_start(u_d.ap()[:, 2 * QW * p: 2 * QW * (p + 1)], U[:])

                    # pipelined emission: produce kv chunks as x streams in,
                    # interleave q-projection and attention so ACT/DVE start
                    # early and PE never waits on a whole phase.
                    for j in range(4):
                        emit_kv_chunk(j)
                        emit_q(2 * j)
                        emit_q(2 * j + 1)
                        emit_attn(2 * j)
                        emit_attn(2 * j + 1)
                    for t in range(4, 8):
                        emit_q(t)
                    for p in range(8, NBLK):
                        emit_attn(p)
    nc.compile()
    return nc


def _perm(role):
    """Per-core tile permutation: slot i holds x tile perm[i] (involution)."""
    idx = np.arange(T // P)
    return idx ^ role


def _make_core_inputs(x, Wq, Wk, Wv):
    w_all = np.concatenate([Wq.T / SCALE, Wk.T, Wv.T], axis=1)  # [C, 3H]
    w_all = np.ascontiguousarray(w_all).astype(BF16)
    tri = np.triu(np.ones((P, P), np.float32))
    in_maps = []
    for c in range(8):
        b, r = c // 2, c % 2
        rows = (np.arange(T).reshape(T // P, P)[_perm(r)]).ravel()
        xT = np.ascontiguousarray(x[b][rows].T).astype(BF16)
        masks = np.empty((P, QW), np.float32)
        masks[:, :P] = tri
        masks[:, P:] = 1.0 if r == 0 else 0.0
        in_maps.append(dict(xT=xT, w_all=w_all, masks=masks.astype(BF16)))
    return in_maps


def _merge_outputs(res):
    """Host epilogue: unswap q order, add pair partials, divide."""
    full = np.empty((B, T, H), np.float32)
    for b in range(B):
        num = np.zeros((T, H), np.float32)
        den = np.zeros((T,), np.float32)
        for r in range(2):
            out = res[2 * b + r]
            nT = np.asarray(out["outT"], np.float32)        # [H, T] core-q-order
            u = np.asarray(out["u"], np.float32)            # [P, NBLK*2*QW]
            d = np.empty((T,), np.float32)
            for p in range(NBLK):
                ub = u[:, 2 * QW * p: 2 * QW * (p + 1)]
                dp = ub[:, :QW].sum(0)
                if p >= 1:
                    dp = dp + ub[:, QW:].sum(0)
                d[QW * p: QW * (p + 1)] = dp
            pm = _perm(r)
            num += nT.T.reshape(T // P, P, H)[pm].reshape(T, H)
            den += d.reshape(T // P, P)[pm].reshape(T)
        full[b] = num / den[:, None]
    return full


def kernel(x, Wq, Wk, Wv):
    x = np.asarray(x, dtype=np.float32)
    if "nc" not in _prog_cache:
        _prog_cache["nc"] = _build_program()
    nc = _prog_cache["nc"]
    in_maps = _make_core_inputs(
        x, np.asarray(Wq, np.float32), np.asarray(Wk, np.float32),
        np.asarray(Wv, np.float32)
    )
    res = run_bass_kernel_spmd(nc, in_maps, list(range(8))).results
    return _merge_outputs(res)


def _mock_device(in_map):
    """Numpy emulation of the device program (fp32; validates indexing)."""
    xT = np.asarray(in_map["xT"], np.float32)       # [C, T] permuted
    w = np.asarray(in_map["w_all"], np.float32)     # [C, 3H]
    masks = np.asarray(in_map["masks"], np.float32)  # [P, QW]
    wq, wk, wv = w[:, :H], w[:, H:2 * H], w[:, 2 * H:]
    qT = wq.T @ xT                                   # [H, T]
    kT = np.concatenate(
        [wk.T @ xT[:, 2 * P * pos: 2 * P * pos + P] for pos in range(NPOS)], axis=1)
    v = np.concatenate(
        [xT[:, 2 * P * pos: 2 * P * pos + P].T @ wv for pos in range(NPOS)], axis=0
    ).reshape(NPOS, P, H)
    outT = np.zeros((H, T), np.float32)
    u_out = np.zeros((P, NBLK * 2 * QW), np.float32)
    for p in range(NBLK):
        qs = qT[:, QW * p: QW * (p + 1)]
        po = np.zeros((H, QW), np.float32)
        U = np.zeros((P, 2 * QW), np.float32)
        for j in range(p + 1):
            sT = kT[:, P * j: P * (j + 1)].T @ qs    # [kv, q]
            pj = np.exp(sT)
            if j == p:
                pj = pj * masks
            U[:, QW * (j % 2): QW * (j % 2) + QW] += pj
            po += v[j].T @ pj
        outT[:, QW * p: QW * (p + 1)] = po
        u_out[:, 2 * QW * p: 2 * QW * (p + 1)] = U
    return dict(outT=outT, u=u_out)


def _mock_check():
    rng = np.random.default_rng(0)
    x = rng.standard_normal((B, T, C)).astype(np.float32)
    s = 1.0 / np.sqrt(C)
    Wq = rng.uniform(-s, s, (H, C)).astype(np.float32)
    Wk = rng.uniform(-s, s, (H, C)).astype(np.float32)
    Wv = rng.uniform(-s, s, (H, C)).astype(np.float32)
    exp = np.empty((B, T, H), np.float32)
    causal = np.tril(np.ones((T, T), bool))
    for b in range(B):
        q = x[b] @ Wq.T
        k = x[b] @ Wk.T
        vv = x[b] @ Wv.T
        sc = (q @ k.T) / SCALE
        sc = np.where(causal, sc, -np.inf)
        sc = sc - sc.max(1, keepdims=True)
        a = np.exp(sc)
        a /= a.sum(1, keepdims=True)
        exp[b] = a @ vv
    in_maps = _make_core_inputs(x, Wq, Wk, Wv)
    res = [_mock_device(m) for m in in_maps]
    act = _merge_outputs(res)
    rel = np.linalg.norm(act - exp) / np.linalg.norm(exp)
    print(f"mock rel err: {rel:.4e}  max abs: {np.abs(act - exp).max():.3e}")
    assert rel < 2e-2, "mock check failed"


if __name__ == "__main__":
    if "--mock" in sys.argv:
        _mock_check()
    else:
        nc = _build_program()
        print("program built ok")
